# revision 1
# baseline (speedup 1.0000x reference)
"""CombinedCSA (channel+spatial attention) Trainium2 Bass kernel.

Sharding: data-parallel over batch. 16 images / 8 cores = 2 images per core.
Weights (fc1/fc2/conv) replicated, pre-transposed host-side.

Per-core dataflow (per image, streamed in HW chunks of 16 rows):
  load chunk -> channel-max (DVE reduce) + channel-sum (ACT accum_out)
  MLP (PE matmuls + ACT relu/sigmoid) -> per-channel scale
  scale chunk in place (ACT, per-partition scale)
  spatial max over C: DVE max(half0,half1) -> PE transpose -> DVE segmented reduce
  spatial sum over C: PE matmul (x block stationary, ones moving)
  7x7 conv: 14 banded matmuls on PE (bands built host-side)
  sigmoid -> transpose -> row-collapse DMA -> gpsimd partition_broadcast
  final multiply in place (DVE / gpsimd split) -> store
"""

import os
import numpy as np
from contextlib import ExitStack

import concourse.bass as bass
import concourse.tile as tile
from concourse import bacc, mybir
from concourse._compat import with_exitstack
from concourse.bass_utils import run_bass_kernel_spmd

F32 = mybir.dt.float32
AF = mybir.ActivationFunctionType

# Problem constants (hardcoded; see spec)
B, C, H, W = 16, 256, 128, 128
HW = H * W          # 16384
R = 16              # Cr = C // 16
NCORES = 8
BLOC = B // NCORES  # 2 images per core
NH = 2              # channel halves of 128
P = 128
FCH = 2048          # hw elements per chunk (16 h-rows)
NCH = HW // FCH     # 8 chunks per image
HROWS = FCH // W    # 16 h-rows per chunk
CONVG = 2           # chunks per conv group
NBLK = FCH // P     # 16 transpose blocks per chunk

# chunk indices whose heavy elementwise ops go to gpsimd instead of DVE
# (walrus rejects TensorTensor on the Pool engine on this toolchain, so empty)
GPS_FINAL = frozenset()
GPS_COMBINE = frozenset()


@with_exitstack
def csa_kernel(ctx, tc, out_d, x_d, w1t_d, w2t_d, bands_d, ident_d,
               skip=frozenset()):
    nc = tc.nc

    # ---- pools ----
    xp = ctx.enter_context(tc.tile_pool(name="xp", bufs=19))
    xmaxp = ctx.enter_context(tc.tile_pool(name="xmaxp", bufs=2))
    bcp = ctx.enter_context(tc.tile_pool(name="bcp", bufs=1))
    rowp = ctx.enter_context(tc.tile_pool(name="rowp", bufs=1))
    stat = ctx.enter_context(tc.tile_pool(name="stat", bufs=2))
    cons = ctx.enter_context(tc.tile_pool(name="cons", bufs=1))
    tp = ctx.enter_context(tc.tile_pool(name="tp", bufs=2, space="PSUM"))
    tsp = ctx.enter_context(tc.tile_pool(name="tsp", bufs=2, space="PSUM"))
    convp = ctx.enter_context(tc.tile_pool(name="convp", bufs=2, space="PSUM"))
    atpp = ctx.enter_context(tc.tile_pool(name="atpp", bufs=1, space="PSUM"))
    mlpp = ctx.enter_context(tc.tile_pool(name="mlpp", bufs=1, space="PSUM"))

    # ---- constants / weights ----
    w1t_sb = cons.tile([P, NH * R], F32)           # [128, 32]: col block h = w_fc1.T half h
    for h in range(NH):
        nc.sync.dma_start(out=w1t_sb[:, h * R:(h + 1) * R],
                          in_=w1t_d[h * P:(h + 1) * P, :])
    w2t_sb = cons.tile([R, C], F32)                # [16, 256] = w_fc2.T
    nc.sync.dma_start(out=w2t_sb[:], in_=w2t_d[:])
    bands_sb = cons.tile([P, 14 * P], F32)         # [128, (ci, w)]
    nc.sync.dma_start(out=bands_sb[:].rearrange("p (c w) -> p c w", c=14),
                      in_=bands_d.transpose([1, 0, 2]))
    ident_sb = cons.tile([P, P], F32)
    nc.sync.dma_start(out=ident_sb[:], in_=ident_d[:])
    ones_sb = cons.tile([P, 1], F32)
    nc.vector.memset(ones_sb[:], 1.0)

    for b in range(BLOC):
        # ---------- phase A: load + channel pooling ----------
        xt = [[None] * NCH for _ in range(NH)]
        chmax_p = []
        chsum_p = []
        for h in range(NH):
            cmp_t = stat.tile([P, NCH], F32, name=f"chmaxp{b}{h}", tag=f"chmaxp{h}")
            csp_t = stat.tile([P, NCH], F32, name=f"chsump{b}{h}", tag=f"chsump{h}")
            chmax_p.append(cmp_t)
            chsum_p.append(csp_t)
            if "chpool" in skip:
                nc.vector.memset(cmp_t[:], 0.5)
                nc.vector.memset(csp_t[:], 0.5)
        for k in range(NCH):
            for h in range(NH):
                t = xp.tile([P, FCH], F32, name=f"x{b}{h}{k}", tag="x")
                xt[h][k] = t
                nc.sync.dma_start(
                    out=t[:],
                    in_=x_d[b, h * P:(h + 1) * P, k * FCH:(k + 1) * FCH])
                if "chpool" in skip:
                    continue
                nc.vector.tensor_reduce(
                    out=chmax_p[h][:, k:k + 1], in_=t[:],
                    axis=mybir.AxisListType.X, op=mybir.AluOpType.max)
                # in-place copy whose only purpose is the free-dim sum output
                nc.scalar.activation(
                    out=t[:], in_=t[:], func=AF.Copy,
                    accum_out=chsum_p[h][:, k:k + 1])

        # ---------- phase B: channel-attention MLP ----------
        scale_sb = []
        z_ps = mlpp.tile([R, 1], F32, name=f"zps{b}", tag="mlp")
        hvec = []
        for h in range(NH):
            cmf = stat.tile([P, 1], F32, name=f"chmaxf{b}{h}", tag=f"chmaxf{h}")
            csf = stat.tile([P, 1], F32, name=f"chsumf{b}{h}", tag=f"chsumf{h}")
            nc.vector.tensor_reduce(out=cmf[:], in_=chmax_p[h][:],
                                    axis=mybir.AxisListType.X,
                                    op=mybir.AluOpType.max)
            nc.vector.tensor_reduce(out=csf[:], in_=chsum_p[h][:],
                                    axis=mybir.AxisListType.X,
                                    op=mybir.AluOpType.add)
            hv = stat.tile([P, 1], F32, name=f"hvec{b}{h}", tag=f"hvec{h}")
            # hv = chmax + chsum/HW
            nc.scalar.activation(out=hv[:], in_=csf[:], func=AF.Identity,
                                 bias=cmf[:, 0:1], scale=1.0 / HW)
            hvec.append(hv)
        for h in range(NH):
            nc.tensor.matmul(out=z_ps[:], lhsT=w1t_sb[:, h * R:(h + 1) * R],
                             rhs=hvec[h][:], start=(h == 0), stop=(h == NH - 1))
        zr = stat.tile([R, 1], F32, name=f"zrelu{b}", tag="zrelu")
        nc.scalar.activation(out=zr[:], in_=z_ps[:], func=AF.Relu)
        for h in range(NH):
            l_ps = mlpp.tile([P, 1], F32, name=f"lps{b}{h}", tag="mlp")
            nc.tensor.matmul(out=l_ps[:], lhsT=w2t_sb[:, h * P:(h + 1) * P],
                             rhs=zr[:], start=True, stop=True)
            sc = stat.tile([P, 1], F32, name=f"scale{b}{h}", tag=f"scale{h}")
            nc.scalar.activation(out=sc[:], in_=l_ps[:], func=AF.Sigmoid)
            scale_sb.append(sc)

        # ---------- phase C/D/E: scale, spatial stats, conv, final ----------
        smaxT = stat.tile([P, H], F32, name=f"smaxT{b}", tag="smaxT")   # [w, h]
        savgT = stat.tile([P, H], F32, name=f"savgT{b}", tag="savgT")   # [w, h]
        conv_ps = convp.tile([P, H], F32, name=f"convps{b}", tag="conv")
        bcs = {}
        if "trans" in skip:
            nc.vector.memset(smaxT[:], 0.25)
        if "savg" in skip:
            nc.vector.memset(savgT[:], 0.25)

        def conv_pair(g):
            h0c, h1c = g * CONVG * HROWS, (g + 1) * CONVG * HROWS
            # 7x7 conv as banded matmuls: out[:, h] += bandT_{c,i} @ statT[:, h+i-3]
            mms = []
            for c, st in ((0, smaxT), (1, savgT)):
                for i in range(7):
                    lo = max(h0c, 3 - i)
                    hi = min(h1c, H + 3 - i)
                    if lo >= hi:
                        continue
                    mms.append((c, i, lo, hi, st))
            # identity-shift tap first so start=True covers the whole column range
            mms.sort(key=lambda m: (m[1] != 3 or m[0] != 0))
            for n, (c, i, lo, hi, st) in enumerate(mms):
                assert not (n == 0 and (lo != h0c or hi != h1c))
                nc.tensor.matmul(
                    out=conv_ps[:, lo:hi],
                    lhsT=bands_sb[:, (c * 7 + i) * P:(c * 7 + i + 1) * P],
                    rhs=st[:, lo + i - 3:hi + i - 3],
                    start=(n == 0), stop=(n == len(mms) - 1),
                    skip_group_check=True)

        def attn_chunk(kc):
            h0c, h1c = kc * HROWS, (kc + 1) * HROWS
            attn_wh = stat.tile([P, HROWS], F32, name=f"attnwh{b}{kc}",
                                tag="attnwh", bufs=3)
            nc.scalar.activation(out=attn_wh[:], in_=conv_ps[:, h0c:h1c],
                                 func=AF.Sigmoid)
            at_ps = atpp.tile([HROWS, P], F32, name=f"atps{b}{kc}", tag="atp")
            nc.tensor.transpose(out=at_ps[:], in_=attn_wh[:], identity=ident_sb[:])
            attn_hw = stat.tile([HROWS, P], F32, name=f"attnhw{b}{kc}",
                                tag="attnhw", bufs=3)
            nc.scalar.activation(out=attn_hw[:], in_=at_ps[:], func=AF.Copy)
            row = rowp.tile([1, FCH], F32, name=f"row{b}{kc}", tag="row")
            nc.sync.dma_start(
                out=row[:].rearrange("p (h w) -> p h w", h=HROWS),
                in_=attn_hw[:])
            bc = bcp.tile([P, FCH], F32, name=f"bc{b}{kc}", tag="bc")
            nc.gpsimd.partition_broadcast(bc[:], row[:], channels=P)
            bcs[kc] = bc

        def conv_and_final(g):
            if "conv" not in skip:
                conv_pair(g)
            for kc in range(CONVG * g, CONVG * (g + 1)):
                if "conv" not in skip:
                    attn_chunk(kc)
                for h in range(NH):
                    if "final" not in skip and "conv" not in skip:
                        nc.vector.tensor_mul(xt[h][kc][:], xt[h][kc][:],
                                             bcs[kc][:])
                    nc.sync.dma_start(
                        out=out_d[b, h * P:(h + 1) * P,
                                  kc * FCH:(kc + 1) * FCH],
                        in_=xt[h][kc][:])

        for k in range(NCH):
            if "scale" not in skip:
                for h in range(NH):
                    nc.scalar.activation(out=xt[h][k][:], in_=xt[h][k][:],
                                         func=AF.Copy,
                                         scale=scale_sb[h][:, 0:1])
            # spatial max over C: combine halves, transpose, segmented reduce.
            # spatial sum over C: transpose both halves into the same PSUM
            # region with accumulation, then segmented add-reduce.
            if "trans" not in skip:
                xm = xmaxp.tile([P, FCH], F32, name=f"xm{b}{k}", tag="xm")
                nc.vector.tensor_max(xm[:], xt[0][k][:], xt[1][k][:])
                for j4 in range(NBLK // 4):
                    tpt = tp.tile([P, 4 * P], F32, name=f"tp{b}{k}{j4}",
                                  tag="tp")
                    tps = tsp.tile([P, 4 * P], F32, name=f"ts{b}{k}{j4}",
                                   tag="ts")
                    for jj in range(4):
                        j = j4 * 4 + jj
                        nc.tensor.transpose(out=tpt[:, jj * P:(jj + 1) * P],
                                            in_=xm[:, j * P:(j + 1) * P],
                                            identity=ident_sb[:])
                        if "savg" in skip:
                            continue
                        nc.tensor.matmul(out=tps[:, jj * P:(jj + 1) * P],
                                         lhsT=xt[0][k][:, j * P:(j + 1) * P],
                                         rhs=ident_sb[:], is_transpose=True,
                                         start=True, stop=False,
                                         skip_group_check=True)
                        nc.tensor.matmul(out=tps[:, jj * P:(jj + 1) * P],
                                         lhsT=xt[1][k][:, j * P:(j + 1) * P],
                                         rhs=ident_sb[:], is_transpose=True,
                                         start=False, stop=True,
                                         skip_group_check=True)
                    g0 = k * NBLK + j4 * 4
                    nc.vector.tensor_reduce(
                        out=smaxT[:, g0:g0 + 4],
                        in_=tpt[:].rearrange("p (b f) -> p b f", b=4),
                        axis=mybir.AxisListType.X, op=mybir.AluOpType.max)
                    if "savg" not in skip:
                        nc.vector.tensor_reduce(
                            out=savgT[:, g0:g0 + 4],
                            in_=tps[:].rearrange("p (b f) -> p b f", b=4),
                            axis=mybir.AxisListType.X, op=mybir.AluOpType.add)
            if k >= CONVG and k % CONVG == 0:
                conv_and_final((k - CONVG) // CONVG)
        conv_and_final(NCH // CONVG - 1)


def _build_nc(reps: int = 1, skip=frozenset()):
    nc = bacc.Bacc("TRN2", target_bir_lowering=False, debug=False,
                   num_devices=NCORES)
    x_d = nc.dram_tensor("x", [BLOC, C, HW], F32, kind="ExternalInput").ap()
    w1t_d = nc.dram_tensor("w1t", [C, R], F32, kind="ExternalInput").ap()
    w2t_d = nc.dram_tensor("w2t", [R, C], F32, kind="ExternalInput").ap()
    bands_d = nc.dram_tensor("bands", [14, W, W], F32, kind="ExternalInput").ap()
    ident_d = nc.dram_tensor("ident", [P, P], F32, kind="ExternalInput").ap()
    out_d = nc.dram_tensor("out", [BLOC, C, HW], F32, kind="ExternalOutput").ap()
    with tile.TileContext(nc) as tc:
        for _ in range(reps):
            csa_kernel(tc, out_d, x_d, w1t_d, w2t_d, bands_d, ident_d,
                       skip=skip)
    nc.compile()
    return nc


_NC_CACHE = None


def _get_nc():
    global _NC_CACHE
    if _NC_CACHE is None:
        _NC_CACHE = _build_nc()
    return _NC_CACHE


def build_bands(w_conv):
    """[14, W, W] transposed band matrices; bands[c*7+i][w', w] =
    w_conv[0, c, i, w'-w+3]; avg channel folded with 1/C."""
    w_conv = np.asarray(w_conv, np.float32)
    bands = np.zeros((2, 7, W, W), np.float32)
    for c in range(2):
        for i in range(7):
            for kj in range(7):
                bands[c, i] += w_conv[0, c, i, kj] * np.eye(W, k=3 - kj,
                                                            dtype=np.float32)
    bands[1] /= C
    return bands.reshape(14, W, W)


def make_in_maps(x, w_fc1, w_fc2, w_conv):
    x = np.ascontiguousarray(np.asarray(x, np.float32))
    w1t = np.ascontiguousarray(np.asarray(w_fc1, np.float32).T)
    w2t = np.ascontiguousarray(np.asarray(w_fc2, np.float32).T)
    bands = build_bands(w_conv)
    ident = np.eye(P, dtype=np.float32)
    xr = x.reshape(NCORES, BLOC, C, HW)
    return [{"x": np.ascontiguousarray(xr[i]), "w1t": w1t, "w2t": w2t,
             "bands": bands, "ident": ident} for i in range(NCORES)]


def kernel(x, w_fc1, w_fc2, w_conv):
    nc = _get_nc()
    in_maps = make_in_maps(x, w_fc1, w_fc2, w_conv)
    res = run_bass_kernel_spmd(nc, in_maps, list(range(NCORES)))
    out = np.stack([res.results[i]["out"] for i in range(NCORES)])
    return out.reshape(B, C, H, W).astype(np.float32)



# revision 15
# speedup vs baseline: 1.9224x; 1.9224x over previous
"""CombinedCSA (channel+spatial attention) Trainium2 Bass kernel, bf16.

Sharding: data-parallel over batch. 16 images / 8 cores = 2 images per core.
Weights (fc1/fc2/conv) replicated, pre-transposed host-side. x is downcast
to bf16 host-side; output is stored bf16 and upcast host-side (rel-err
budget 2e-2 >> bf16 rounding).

Per-core dataflow (per image):
  A. stream 16 half-chunks [128, 2048] in; DVE running-fold channel-max;
     PE matmul w1 @ x accumulated into PSUM (= fc1 of the mean, folded)
  B. MLP: PE w1@chmax + ACT relu(+mean bias) + PE fc2 + ACT sigmoid -> s
  C. per chunk: scale halves in place (DVE tensor_scalar / ACT copy-scale),
     DVE max-combine halves, gpsimd partition_all_reduce(max) -> spatial max
     row, row DMA into [h, w] stat tile; PE one-hot matmuls -> spatial sum
     in [h, w] layout
  D. 7x7 conv as 14 banded matmuls on PE (bands shift along w, [h,w] layout),
     ACT sigmoid -> attn
  E. per chunk: attn row DMA, PE outer-product broadcast (ones x row) to
     PSUM, ACT evac to bf16, DVE multiply in place, store
"""

import numpy as np
import ml_dtypes
from contextlib import ExitStack

import concourse.bass as bass
import concourse.tile as tile
from concourse import bacc, mybir, bass_isa
from concourse._compat import with_exitstack
from concourse.bass_utils import run_bass_kernel_spmd

F32 = mybir.dt.float32
BF16 = mybir.dt.bfloat16
AF = mybir.ActivationFunctionType
ALU = mybir.AluOpType

# Problem constants (hardcoded; see spec)
B, C, H, W = 16, 256, 128, 128
HW = H * W          # 16384
R = 16              # Cr = C // 16
NCORES = 8
BLOC = B // NCORES  # 2 images per core
NH = 2              # channel halves of 128
P = 128
FCH = 2048          # hw elements per chunk (16 h-rows)
NCH = HW // FCH     # 8 chunks per image
HROWS = FCH // W    # 16 h-rows per chunk
FH = 1024           # chmax fold width


@with_exitstack
def make_pools(ctx, tc):
    pools = {}
    pools["xp"] = ctx.enter_context(tc.tile_pool(name="xp", bufs=18))
    pools["mp"] = ctx.enter_context(tc.tile_pool(name="mp", bufs=3))
    pools["mrp"] = ctx.enter_context(tc.tile_pool(name="mrp", bufs=2))
    pools["bcp"] = ctx.enter_context(tc.tile_pool(name="bcp", bufs=2))
    pools["stat"] = ctx.enter_context(tc.tile_pool(name="stat", bufs=2))
    pools["cons"] = ctx.enter_context(tc.tile_pool(name="cons", bufs=1))
    pools["fc1p"] = ctx.enter_context(
        tc.tile_pool(name="fc1p", bufs=1, space="PSUM"))
    pools["svp"] = ctx.enter_context(
        tc.tile_pool(name="svp", bufs=1, space="PSUM"))
    pools["bcps"] = ctx.enter_context(
        tc.tile_pool(name="bcps", bufs=2, space="PSUM"))
    pools["convp"] = ctx.enter_context(
        tc.tile_pool(name="convp", bufs=1, space="PSUM"))
    pools["mlpp"] = ctx.enter_context(
        tc.tile_pool(name="mlpp", bufs=1, space="PSUM"))
    return pools


def load_consts(tc, pools, w1t_d, w2t_d, bands_d, oh_d):
    nc = tc.nc
    cons = pools["cons"]
    w1t_sb = cons.tile([P, NH * R], BF16)          # [128, 32]: w_fc1.T halves
    for h in range(NH):
        nc.sync.dma_start(out=w1t_sb[:, h * R:(h + 1) * R],
                          in_=w1t_d[h * P:(h + 1) * P, :])
    w2t_sb = cons.tile([R, C], BF16)               # [16, 256] = w_fc2.T
    nc.sync.dma_start(out=w2t_sb[:], in_=w2t_d[:])
    bands_sb = cons.tile([P, 14 * P], BF16)        # [h, (c*7+j, h')]
    nc.sync.dma_start(out=bands_sb[:].rearrange("p (c w) -> p c w", c=14),
                      in_=bands_d.transpose([1, 0, 2]))
    oh_sb = cons.tile([P, 2 * W], BF16)            # all-ones column at index W
    nc.sync.dma_start(out=oh_sb[:], in_=oh_d[:])
    ones_sb = cons.tile([1, P], BF16)
    nc.vector.memset(ones_sb[:], 1.0)
    return dict(w1t_sb=w1t_sb, w2t_sb=w2t_sb, bands_sb=bands_sb,
                oh_sb=oh_sb, ones_sb=ones_sb)


def csa_kernel(tc, pools, consts, rep, out_d, x_d, skip=frozenset()):
    nc = tc.nc
    xp, mp, mrp, bcp, stat = (pools[k] for k in
                              ("xp", "mp", "mrp", "bcp", "stat"))
    fc1p, svp, bcps, convp, mlpp = (pools[k] for k in
                                    ("fc1p", "svp", "bcps", "convp", "mlpp"))
    w1t_sb = consts["w1t_sb"]
    w2t_sb = consts["w2t_sb"]
    bands_sb = consts["bands_sb"]
    oh_sb = consts["oh_sb"]
    ones_sb = consts["ones_sb"]

    for bb in range(BLOC):
        b = bb
        r_ = rep
        # ---------- phase A: load + channel stats ----------
        # x held as NCH//2 double-width tiles per half (fewer DMA instrs)
        xtt = [[None] * (NCH // 2) for _ in range(NH)]
        xt = [[None] * NCH for _ in range(NH)]
        runm = [stat.tile([P, FH], BF16, name=f"runm{r_}_{b}{h}", tag=f"runm{h}")
                for h in range(NH)]
        fc1_ps = fc1p.tile([R, 512], F32, name=f"fc1ps{r_}_{b}", tag="fc1")
        for kk in range(NCH // 2):
            for h in range(NH):
                t = xp.tile([P, 2 * FCH], BF16, name=f"x{r_}_{b}{h}{kk}",
                            tag="x")
                xtt[h][kk] = t
                xt[h][2 * kk] = t[:, 0:FCH]
                xt[h][2 * kk + 1] = t[:, FCH:2 * FCH]
                nc.sync.dma_start(
                    out=t[:],
                    in_=x_d[b, h * P:(h + 1) * P,
                            2 * kk * FCH:(2 * kk + 2) * FCH])
                if "chpool" not in skip:
                    for i in range(2):
                        k = 2 * kk + i
                        tv = xt[h][k]
                        if k == 0:
                            nc.vector.tensor_max(runm[h][:], tv[:, 0:FH],
                                                 tv[:, FH:FCH])
                        else:
                            nc.vector.tensor_max(runm[h][:], runm[h][:],
                                                 tv[:, 0:FH])
                            nc.vector.tensor_max(runm[h][:], runm[h][:],
                                                 tv[:, FH:FCH])
                    for j in range(8):
                        nc.tensor.matmul(
                            out=fc1_ps[:],
                            lhsT=w1t_sb[:, h * R:(h + 1) * R],
                            rhs=t[:, j * 512:(j + 1) * 512],
                            start=(kk == 0 and h == 0 and j == 0),
                            stop=(kk == NCH // 2 - 1 and h == NH - 1
                                  and j == 7),
                            skip_group_check=True)

        # ---------- phase B: channel-attention MLP ----------
        s_f32 = []
        mlp_ps = mlpp.tile([P, 4], F32, name=f"mlps{r_}_{b}", tag="mlp")
        z_ps = mlp_ps[0:R, 0:1]
        mean16 = stat.tile([R, 1], F32, name=f"mean{r_}_{b}", tag="mean")
        trash = stat.tile([R, 512], BF16, name=f"trash{r_}_{b}", tag="trash")
        if "chpool" in skip:
            nc.vector.memset(mean16[:], 0.1)
        else:
            nc.scalar.activation(out=trash[:], in_=fc1_ps[:], func=AF.Copy,
                                 scale=1.0 / HW, accum_out=mean16[:])
        for h in range(NH):
            cm = stat.tile([P, 1], BF16, name=f"cm{r_}_{b}{h}", tag=f"cm{h}")
            if "chpool" in skip:
                nc.vector.memset(cm[:], 0.5)
            else:
                nc.vector.tensor_reduce(out=cm[:], in_=runm[h][:],
                                        axis=mybir.AxisListType.X,
                                        op=ALU.max)
            nc.tensor.matmul(out=z_ps, lhsT=w1t_sb[:, h * R:(h + 1) * R],
                             rhs=cm[:], start=(h == 0), stop=(h == NH - 1),
                             skip_group_check=True)
        zr = stat.tile([R, 1], BF16, name=f"zr{r_}_{b}", tag="zr")
        nc.scalar.activation(out=zr[:], in_=z_ps, func=AF.Relu,
                             bias=mean16[:, 0:1])
        for h in range(NH):
            l_ps = mlp_ps[:, 1 + h:2 + h]
            nc.tensor.matmul(out=l_ps, lhsT=w2t_sb[:, h * P:(h + 1) * P],
                             rhs=zr[:], start=True, stop=True,
                             skip_group_check=True)
            sc = stat.tile([P, 1], F32, name=f"s{r_}_{b}{h}", tag=f"s{h}")
            nc.scalar.activation(out=sc[:], in_=l_ps, func=AF.Sigmoid)
            s_f32.append(sc)

        # ---------- phase C: scale + spatial stats ----------
        smaxHW = stat.tile([P, W], BF16, name=f"smax{r_}_{b}", tag="smax")
        savgHW = stat.tile([P, W], BF16, name=f"savg{r_}_{b}", tag="savg")
        sv_ps = svp.tile([P, W], F32, name=f"svps{r_}_{b}", tag="sv")
        if "trans" in skip:
            nc.vector.memset(smaxHW[:], 0.25)
        if "savg" in skip:
            nc.vector.memset(savgHW[:], 0.25)
        for k in range(NCH):
            if "scale" not in skip:
                # scale both halves in place: DVE for h0 (4x mode), ACT h1
                nc.vector.tensor_scalar_mul(xt[0][k], xt[0][k],
                                            s_f32[0][:, 0:1])
                nc.scalar.activation(out=xt[1][k], in_=xt[1][k],
                                     func=AF.Copy, scale=s_f32[1][:, 0:1])
            if "trans" not in skip:
                m = mp.tile([P, FCH], BF16, name=f"m{r_}_{b}{k}", tag="m")
                nc.vector.tensor_max(m[:], xt[0][k], xt[1][k])
                mr = mrp.tile([P, FCH], BF16, name=f"mr{r_}_{b}{k}", tag="mr")
                nc.gpsimd.partition_all_reduce(mr[:], m[:], channels=P,
                                               reduce_op=bass_isa.ReduceOp.max)
                nc.sync.dma_start(
                    out=smaxHW[k * HROWS:(k + 1) * HROWS, :],
                    in_=mr[0:1, :].rearrange("p (h w) -> p h w", h=HROWS))
            if "savg" not in skip:
                for j in range(HROWS):
                    q = k * HROWS + j
                    for h in range(NH):
                        nc.tensor.matmul(
                            out=sv_ps[:],
                            lhsT=oh_sb[:, W - q:2 * W - q],
                            rhs=xt[h][k][:, j * W:(j + 1) * W] if False else xt[h][k].slice_free(j * W, (j + 1) * W),
                            start=(k == 0 and j == 0 and h == 0),
                            stop=(k == NCH - 1 and j == HROWS - 1
                                  and h == NH - 1),
                            skip_group_check=True)

        # ---------- phase D: conv + sigmoid ----------
        if "savg" not in skip:
            nc.scalar.activation(out=savgHW[:], in_=sv_ps[:], func=AF.Copy)
        attn = stat.tile([P, W], BF16, name=f"attn{r_}_{b}", tag="attn")
        if "conv" not in skip:
            conv_ps = convp.tile([P, W], F32, name=f"convps{r_}_{b}", tag="conv")
            mms = []
            for c, st in ((0, smaxHW), (1, savgHW)):
                for j in range(7):
                    lo = max(0, 3 - j)
                    hi = min(W, W + 3 - j)
                    mms.append((c, j, lo, hi, st))
            mms.sort(key=lambda mm: (mm[1] != 3 or mm[0] != 0))
            for n, (c, j, lo, hi, st) in enumerate(mms):
                nc.tensor.matmul(
                    out=conv_ps[:, lo:hi],
                    lhsT=bands_sb[:, (c * 7 + j) * P:(c * 7 + j + 1) * P],
                    rhs=st[:, lo + j - 3:hi + j - 3],
                    start=(n == 0), stop=(n == len(mms) - 1),
                    skip_group_check=True)
            nc.scalar.activation(out=attn[:], in_=conv_ps[:], func=AF.Sigmoid)

        # ---------- phase E: broadcast + final multiply + store ----------
        for k in range(NCH):
            if "conv" not in skip and "final" not in skip:
                row = stat.tile([1, FCH], BF16, name=f"row{r_}_{b}{k}", tag="row",
                                bufs=3)
                nc.sync.dma_start(
                    out=row[:].rearrange("p (h w) -> p h w", h=HROWS),
                    in_=attn[k * HROWS:(k + 1) * HROWS, :])
                bc_sb = bcp.tile([P, FCH], BF16, name=f"bc{r_}_{b}{k}", tag="bc")
                for j in range(2):
                    bc_ps = bcps.tile([P, FH], F32, name=f"bcps{r_}_{b}{k}{j}",
                                      tag="bcps")
                    for jj in range(2):
                        nc.tensor.matmul(
                            out=bc_ps[:, jj * 512:(jj + 1) * 512],
                            lhsT=ones_sb[:],
                            rhs=row[0:1, j * FH + jj * 512:
                                     j * FH + (jj + 1) * 512],
                            start=True, stop=True, skip_group_check=True)
                    nc.scalar.activation(out=bc_sb[:, j * FH:(j + 1) * FH],
                                         in_=bc_ps[:], func=AF.Copy)
                for h in range(NH):
                    nc.vector.tensor_mul(xt[h][k], xt[h][k], bc_sb[:])
            if k % 2 == 1:
                for h in range(NH):
                    nc.scalar.dma_start(
                        out=out_d[b, h * P:(h + 1) * P,
                                  (k - 1) * FCH:(k + 1) * FCH],
                        in_=xtt[h][k // 2][:])


def _build_nc(reps: int = 1, skip=frozenset()):
    nc = bacc.Bacc("TRN2", target_bir_lowering=False, debug=False,
                   num_devices=NCORES)
    x_d = nc.dram_tensor("x", [BLOC, C, HW], BF16, kind="ExternalInput").ap()
    w1t_d = nc.dram_tensor("w1t", [C, R], BF16, kind="ExternalInput").ap()
    w2t_d = nc.dram_tensor("w2t", [R, C], BF16, kind="ExternalInput").ap()
    bands_d = nc.dram_tensor("bands", [14, W, W], BF16,
                             kind="ExternalInput").ap()
    oh_d = nc.dram_tensor("oh", [P, 2 * W], BF16,
                          kind="ExternalInput").ap()
    out_d = nc.dram_tensor("out", [BLOC, C, HW], BF16,
                           kind="ExternalOutput").ap()
    with tile.TileContext(nc) as tc:
        with ExitStack() as ctx:
            pools = make_pools.__wrapped__(ctx, tc)
            consts = load_consts(tc, pools, w1t_d, w2t_d, bands_d, oh_d)
            for rep in range(reps):
                csa_kernel(tc, pools, consts, rep, out_d, x_d, skip=skip)
    nc.compile()
    return nc


_NC_CACHE = None


def _get_nc():
    global _NC_CACHE
    if _NC_CACHE is None:
        _NC_CACHE = _build_nc()
    return _NC_CACHE


def build_bands(w_conv):
    """[14, H, H] band matrices for the [h, w] conv formulation:
    bands[c*7+j][h, h'] = w_conv[0, c, h-h'+3, j]; avg channel folded 1/C."""
    w_conv = np.asarray(w_conv, np.float32)
    bands = np.zeros((2, 7, H, H), np.float32)
    for c in range(2):
        for j in range(7):
            for i in range(7):
                bands[c, j] += w_conv[0, c, i, j] * np.eye(H, k=3 - i,
                                                           dtype=np.float32)
    bands[1] /= C
    return bands.reshape(14, H, H)


def build_onehot():
    oh = np.zeros((P, 2 * W), np.float32)
    oh[:, W] = 1.0
    return oh


def make_in_maps(x, w_fc1, w_fc2, w_conv):
    bf = ml_dtypes.bfloat16
    x = np.ascontiguousarray(np.asarray(x)).astype(bf)
    w1t = np.ascontiguousarray(np.asarray(w_fc1, np.float32).T).astype(bf)
    w2t = np.ascontiguousarray(np.asarray(w_fc2, np.float32).T).astype(bf)
    bands = build_bands(w_conv).astype(bf)
    oh = build_onehot().astype(bf)
    xr = x.reshape(NCORES, BLOC, C, HW)
    return [{"x": np.ascontiguousarray(xr[i]), "w1t": w1t, "w2t": w2t,
             "bands": bands, "oh": oh} for i in range(NCORES)]


def kernel(x, w_fc1, w_fc2, w_conv):
    nc = _get_nc()
    in_maps = make_in_maps(x, w_fc1, w_fc2, w_conv)
    res = run_bass_kernel_spmd(nc, in_maps, list(range(NCORES)))
    out = np.stack([np.asarray(res.results[i]["out"]) for i in range(NCORES)])
    return out.reshape(B, C, H, W).astype(np.float32)
